# revision 1
# baseline (speedup 1.0000x reference)
"""CSPN 3x3 propagation step on 8 Trainium2 NeuronCores.

out[b,0,r,c] = sum_k aff[b,k,r,c] * patch_k(cur)[r,c], with the center tap
(k=4) taken from coarse_seg instead of cur_seg. Zero padding at image edges.

Sharding: pure data parallel over batch (16 images -> 2 per core), one SPMD
Bass program run on all 8 cores with per-core input slices.

Per-core algorithm (per 512x512 image, packed as [128 partitions, 4 row
blocks, 512 cols]):
  - The tap row-shift (dy) is folded into the affinity DMA: plane k is
    loaded with a source row offset of -dy_k (A'_k[s] = aff_k[s-dy]).
    The overhanging first/last source row of the shifted window lands in
    an adjacent affinity plane (never out of bounds) and its product is
    provably never consumed.
  - The tap col-shift (dx) is a free-dim offset into a column-padded cur
    tile.
  - VectorEngine computes the 9 elementwise products P_k = A'_k * cur_x,
    then per-dy-group sums V_g (2 adds per group; optionally on GpSimd).
  - TensorEngine realigns the dy groups with shift-matrix matmuls
    (multiply by exact 0/1 -> bit-exact) accumulating in PSUM, including
    the cross-block boundary rows.
  - ScalarEngine evacuates PSUM -> SBUF; DMA stores the result.
"""

import sys

import numpy as np

if "/opt/trn_rl_repo" not in sys.path:
    sys.path.insert(0, "/opt/trn_rl_repo")

B_PER_CORE = 2
N_CORES = 8
H = 512
W = 512
NBLK = H // 128
WPAD = W + 2  # zero column on each side

_compiled = None
_compiled_reps = {}


def _shift_mats():
    """[128, 5, 128] f32: j=0 I, 1 Sd (k=m-1), 2 Su (k=m+1), 3 Ed, 4 Eu."""
    m = np.zeros((128, 5, 128), dtype=np.float32)
    for i in range(128):
        m[i, 0, i] = 1.0  # identity
    for i in range(127):
        m[i, 1, i + 1] = 1.0  # Sd: out[m] = in[m-1]
        m[i + 1, 2, i] = 1.0  # Su: out[m] = in[m+1]
    m[127, 3, 0] = 1.0  # Ed: out[0] = in[127]   (prev block)
    m[0, 4, 127] = 1.0  # Eu: out[127] = in[0]   (next block)
    return m


def _build_program(reps=1):
    """reps>1 unrolls the whole per-core computation `reps` times inside one
    NEFF — used only to measure kernel time through the dispatch noise."""
    import concourse.bacc as bacc
    import concourse.mybir as mybir
    import concourse.tile as tile

    fp32 = mybir.dt.float32

    nc = bacc.Bacc(
        "TRN2",
        target_bir_lowering=False,
        debug=False,
        enable_asserts=False,
        num_devices=N_CORES,
    )

    aff_d = nc.dram_tensor(
        "affinity", [B_PER_CORE, 9, H, W], fp32, kind="ExternalInput"
    ).ap()
    cur_d = nc.dram_tensor(
        "cur_seg", [B_PER_CORE, 1, H, W], fp32, kind="ExternalInput"
    ).ap()
    coa_d = nc.dram_tensor(
        "coarse_seg", [B_PER_CORE, 1, H, W], fp32, kind="ExternalInput"
    ).ap()
    smat_d = nc.dram_tensor("smats", [128, 5, 128], fp32, kind="ExternalInput").ap()
    out_d = nc.dram_tensor(
        "out", [B_PER_CORE, 1, H, W], fp32, kind="ExternalOutput"
    ).ap()

    with tile.TileContext(nc) as tc:
        with (
            tc.tile_pool(name="smat", bufs=1) as smat_pool,
            tc.tile_pool(name="aff", bufs=9) as aff_pool,
            tc.tile_pool(name="prod", bufs=7) as prod_pool,
            tc.tile_pool(name="cur", bufs=2) as cur_pool,
            tc.tile_pool(name="coa", bufs=2) as coa_pool,
            tc.tile_pool(name="acc", bufs=2) as acc_pool,
            tc.tile_pool(name="psum", bufs=8, space="PSUM") as psum_pool,
        ):
            tS = smat_pool.tile([128, 5, 128], fp32)
            SM_I, SM_SD, SM_SU, SM_ED, SM_EU = (tS[:, j, :] for j in range(5))
            smats_loaded = False

            for b in [bb for _ in range(reps) for bb in range(B_PER_CORE)]:
                last_img = b == B_PER_CORE - 1
                # --- cur tile [128, 4, 514], data in cols 1..512 ---
                # cur/coarse ride the ACT HWDGE ring; affinity mostly rides
                # the SP ring, so the two streams overlap.
                tM = cur_pool.tile([128, NBLK, WPAD], fp32, tag="cur")
                nc.vector.memset(tM[:, :, 0:1], 0.0)
                nc.vector.memset(tM[:, :, WPAD - 1 : WPAD], 0.0)
                cur_blocks = cur_d[b, 0].rearrange("(t p) c -> p t c", p=128)
                # split across both rings so cur completes ASAP (gates all
                # products)
                nc.scalar.dma_start(
                    out=tM[:, 0:2, 1 : W + 1], in_=cur_blocks[:, 0:2, :]
                )
                nc.sync.dma_start(
                    out=tM[:, 2:NBLK, 1 : W + 1], in_=cur_blocks[:, 2:NBLK, :]
                )

                # coarse is only needed by the center tap in the dy=0 group
                # (processed last) — defer its load past the dy=+1 planes
                tC = coa_pool.tile([128, NBLK, W], fp32, tag="coa")

                aff_flat = aff_d[b].flatten_outer_dims()  # [9*512, 512]

                acc = acc_pool.tile([128, NBLK, W], fp32, tag="acc")
                out_blocks = out_d[b, 0].rearrange("(t p) c -> p t c", p=128)
                psum_tiles = [
                    psum_pool.tile([128, W], fp32, tag="psum", name=f"ps{b}_{t}")
                    for t in range(NBLK)
                ]

                def _evac_store(t, out_ring):
                    nc.scalar.copy(out=acc[:, t, :], in_=psum_tiles[t])
                    out_ring.dma_start(out=out_blocks[:, t, :], in_=acc[:, t, :])

                def _load_group(g, add_eng, mul0_eng=None, act_dxi=1):
                    """Load the 3 planes of dy-group g (rows shifted -dy),
                    multiply against the shifted cur (or coarse for the
                    center tap), and tree-sum on add_eng. The first product
                    can run on a different engine (mul0_eng) to offload the
                    DVE. Returns V_g."""
                    dy = g - 1
                    Pg = []
                    for dxi in range(3):
                        k = 3 * g + dxi
                        dx = dxi - 1
                        ak = aff_pool.tile([128, NBLK, W], fp32, tag="aff")
                        start = 512 * k - dy
                        ring = nc.scalar if dxi == act_dxi else nc.sync
                        ring.dma_start(
                            out=ak[:],
                            in_=aff_flat[start : start + H, :].rearrange(
                                "(t p) c -> p t c", p=128
                            ),
                        )
                        pk = prod_pool.tile([128, NBLK, W], fp32, tag="prod")
                        src = tC[:] if k == 4 else tM[:, :, 1 + dx : 1 + dx + W]
                        meng = mul0_eng if (dxi == 0 and mul0_eng) else nc.vector
                        meng.tensor_mul(out=pk[:], in0=ak[:], in1=src)
                        Pg.append(pk)
                        if dxi == 1:
                            add_eng.tensor_add(out=Pg[0][:], in0=Pg[0][:], in1=Pg[1][:])
                    add_eng.tensor_add(out=Pg[0][:], in0=Pg[0][:], in1=Pg[2][:])
                    return Pg[0]

                # Groups are processed dy=+1, dy=-1, dy=0: each group's
                # shift-matmuls fire as soon as its sum exists, so by the
                # time the last group (dy=0, plain identity matmuls) lands,
                # the PE queue is nearly drained and the tail is short.
                # psum[t] accumulation order: Su(start), [Eu], Sd, [Ed],
                # I(stop).
                Vp1 = _load_group(2, nc.gpsimd, mul0_eng=nc.gpsimd)
                nc.scalar.dma_start(
                    out=tC[:], in_=coa_d[b, 0].rearrange("(t p) c -> p t c", p=128)
                )
                if not smats_loaded:
                    nc.scalar.dma_start(out=tS[:], in_=smat_d[:])
                    smats_loaded = True
                for t in range(NBLK):
                    nc.tensor.matmul(
                        psum_tiles[t], SM_SU, Vp1[:, t, :], start=True, stop=False
                    )
                    if t < NBLK - 1:
                        nc.tensor.matmul(
                            psum_tiles[t], SM_EU, Vp1[:, t + 1, :],
                            start=False, stop=False,
                        )

                Vm1 = _load_group(0, nc.gpsimd, mul0_eng=nc.gpsimd)
                for t in range(NBLK):
                    nc.tensor.matmul(
                        psum_tiles[t], SM_SD, Vm1[:, t, :], start=False, stop=False
                    )
                    if t > 0:
                        nc.tensor.matmul(
                            psum_tiles[t], SM_ED, Vm1[:, t - 1, :],
                            start=False, stop=False,
                        )

                if not last_img:
                    # --- dy = 0 group, whole-plane path ---
                    V0 = _load_group(1, nc.vector, act_dxi=2)
                    for t in range(NBLK):
                        nc.tensor.matmul(
                            psum_tiles[t], SM_I, V0[:, t, :], start=False, stop=True
                        )
                        _evac_store(t, nc.scalar)
                else:
                    # --- dy = 0 group for the last image: block-halves.
                    # Half 0 (blocks 0-1) loads via ACT while half 1
                    # (blocks 2-3) loads via SP concurrently; psum[0]/[1]
                    # complete as soon as half 0's sum exists, so their
                    # evacuation and stores overlap half 1's compute. The
                    # final serial chain is half-sized.
                    for h in range(2):
                        ring = nc.scalar if h == 0 else nc.sync
                        Ph = []
                        for dxi in range(3):
                            k = 3 + dxi
                            dx = dxi - 1
                            ak = aff_pool.tile([128, 2, W], fp32, tag="aff")
                            start = 512 * k + 256 * h
                            ring.dma_start(
                                out=ak[:],
                                in_=aff_flat[start : start + 256, :].rearrange(
                                    "(t p) c -> p t c", p=128
                                ),
                            )
                            pk = prod_pool.tile([128, 2, W], fp32, tag="prod")
                            src = (
                                tC[:, 2 * h : 2 * h + 2, :]
                                if k == 4
                                else tM[:, 2 * h : 2 * h + 2, 1 + dx : 1 + dx + W]
                            )
                            nc.vector.tensor_mul(out=pk[:], in0=ak[:], in1=src)
                            Ph.append(pk)
                            if dxi == 1:
                                nc.vector.tensor_add(
                                    out=Ph[0][:], in0=Ph[0][:], in1=Ph[1][:]
                                )
                        nc.vector.tensor_add(out=Ph[0][:], in0=Ph[0][:], in1=Ph[2][:])
                        for th in range(2):
                            t = 2 * h + th
                            nc.tensor.matmul(
                                psum_tiles[t], SM_I, Ph[0][:, th, :],
                                start=False, stop=True,
                            )
                            _evac_store(t, nc.scalar if th == 0 else nc.sync)

    nc.compile()
    return nc


def _get_program(reps=1):
    global _compiled
    if reps != 1:
        if reps not in _compiled_reps:
            _compiled_reps[reps] = _build_program(reps)
        return _compiled_reps[reps]
    if _compiled is None:
        _compiled = _build_program()
    return _compiled


def _in_maps(affinity, cur_seg, coarse_seg):
    smats = _shift_mats()
    maps = []
    for j in range(N_CORES):
        s = slice(j * B_PER_CORE, (j + 1) * B_PER_CORE)
        maps.append(
            {
                "affinity": np.ascontiguousarray(affinity[s]),
                "cur_seg": np.ascontiguousarray(cur_seg[s]),
                "coarse_seg": np.ascontiguousarray(coarse_seg[s]),
                "smats": smats,
            }
        )
    return maps


def kernel(affinity, cur_seg, coarse_seg, i=None, **_unused):
    from concourse.bass_utils import run_bass_kernel_spmd

    nc = _get_program()

    affinity = np.ascontiguousarray(affinity, dtype=np.float32)
    cur_seg = np.ascontiguousarray(cur_seg, dtype=np.float32)
    coarse_seg = np.ascontiguousarray(coarse_seg, dtype=np.float32)

    res = run_bass_kernel_spmd(
        nc, _in_maps(affinity, cur_seg, coarse_seg), core_ids=list(range(N_CORES))
    )
    out = np.concatenate([r["out"] for r in res.results], axis=0)
    return out



# revision 3
# speedup vs baseline: 30.3687x; 30.3687x over previous
"""CSPN 3x3 propagation step on 8 Trainium2 NeuronCores.

out[b,0,r,c] = sum_k aff[b,k,r,c] * patch_k(cur)[r,c], with the center tap
(k=4) taken from coarse_seg instead of cur_seg. Zero padding at image edges.

Sharding: pure data parallel over batch (16 images -> 2 per core), one SPMD
Bass program run on all 8 cores with per-core input slices.

Per-core algorithm (per 512x512 image, packed as [128 partitions, 4 row
blocks, 512 cols]):
  - The tap row-shift (dy) is folded into the affinity DMA: plane k is
    loaded with a source row offset of -dy_k (A'_k[s] = aff_k[s-dy]).
    Within a dy-group (3 planes, same dy) the shifted windows are
    contiguous in HBM, so each group loads as ONE 3 MB dma_start
    ([128, 12, 512]; plane j of the group is the [:, 4j:4j+4, :] slice).
    The overhanging first/last source row of the shifted window lands in
    an adjacent affinity plane (never out of bounds) and its product is
    provably never consumed.
  - The tap col-shift (dx) is a free-dim offset into a column-padded cur
    tile.
  - VectorEngine/GpSimd compute the 9 elementwise products P_k = A'_k *
    cur_x and per-dy-group sums V_g; the final add of each group
    downcasts V_g to bf16 (exact-1.0 shift weights, fp32 PSUM accumulate
    -> only the product sums round; rel err ~1e-3, far inside the 2e-2
    gate).
  - TensorEngine realigns the dy groups with bf16 shift-matrix matmuls
    (weights exact 0/1) accumulating in fp32 PSUM, including the
    cross-block boundary rows. bf16 operands keep the PE on the fast
    path (fp32 matmul streams at a fraction of bf16 rate).
  - ScalarEngine evacuates PSUM -> SBUF; DMA stores the result.
"""

import sys

import numpy as np

if "/opt/trn_rl_repo" not in sys.path:
    sys.path.insert(0, "/opt/trn_rl_repo")

B_PER_CORE = 2
N_CORES = 8
H = 512
W = 512
NBLK = H // 128
WPAD = W + 2  # zero column on each side

_compiled = None
_compiled_reps = {}


def _shift_mats():
    """[128, 5, 128] bf16: j=0 I, 1 Sd (k=m-1), 2 Su (k=m+1), 3 Ed, 4 Eu."""
    m = np.zeros((128, 5, 128), dtype=np.float32)
    for i in range(128):
        m[i, 0, i] = 1.0  # identity
    for i in range(127):
        m[i, 1, i + 1] = 1.0  # Sd: out[m] = in[m-1]
        m[i + 1, 2, i] = 1.0  # Su: out[m] = in[m+1]
    m[127, 3, 0] = 1.0  # Ed: out[0] = in[127]   (prev block)
    m[0, 4, 127] = 1.0  # Eu: out[127] = in[0]   (next block)
    from concourse import mybir

    return m.astype(mybir.dt.np(mybir.dt.bfloat16))


def _build_program(reps=1):
    """reps>1 unrolls the whole per-core computation `reps` times inside one
    NEFF — used only to measure kernel time through the dispatch noise."""
    import concourse.bacc as bacc
    import concourse.mybir as mybir
    import concourse.tile as tile

    fp32 = mybir.dt.float32
    bf16 = mybir.dt.bfloat16

    nc = bacc.Bacc(
        "TRN2",
        target_bir_lowering=False,
        debug=False,
        enable_asserts=False,
        num_devices=N_CORES,
    )

    aff_d = nc.dram_tensor(
        "affinity", [B_PER_CORE, 9, H, W], fp32, kind="ExternalInput"
    ).ap()
    cur_d = nc.dram_tensor(
        "cur_seg", [B_PER_CORE, 1, H, W], fp32, kind="ExternalInput"
    ).ap()
    coa_d = nc.dram_tensor(
        "coarse_seg", [B_PER_CORE, 1, H, W], fp32, kind="ExternalInput"
    ).ap()
    smat_d = nc.dram_tensor("smats", [128, 5, 128], bf16, kind="ExternalInput").ap()
    out_d = nc.dram_tensor(
        "out", [B_PER_CORE, 1, H, W], fp32, kind="ExternalOutput"
    ).ap()

    with tile.TileContext(nc) as tc:
        with (
            tc.tile_pool(name="smat", bufs=1) as smat_pool,
            tc.tile_pool(name="aff", bufs=2) as aff_pool,
            tc.tile_pool(name="afft", bufs=7) as afft_pool,
            tc.tile_pool(name="prod", bufs=5) as prod_pool,
            tc.tile_pool(name="vsum", bufs=6) as vsum_pool,
            tc.tile_pool(name="cur", bufs=2) as cur_pool,
            tc.tile_pool(name="coa", bufs=2) as coa_pool,
            tc.tile_pool(name="acc", bufs=2) as acc_pool,
            tc.tile_pool(name="psum", bufs=8, space="PSUM") as psum_pool,
        ):
            tS = smat_pool.tile([128, 5, 128], bf16)
            SM_I, SM_SD, SM_SU, SM_ED, SM_EU = (tS[:, j, :] for j in range(5))
            smats_loaded = False

            for b in [bb for _ in range(reps) for bb in range(B_PER_CORE)]:
                last_img = b == B_PER_CORE - 1
                # --- cur tile [128, 4, 514], data in cols 1..512 ---
                # cur/coarse ride the ACT HWDGE ring; affinity groups are
                # split across both rings so the streams overlap.
                tM = cur_pool.tile([128, NBLK, WPAD], fp32, tag="cur")
                nc.vector.memset(tM[:, :, 0:1], 0.0)
                nc.vector.memset(tM[:, :, WPAD - 1 : WPAD], 0.0)
                cur_blocks = cur_d[b, 0].rearrange("(t p) c -> p t c", p=128)
                # split across both rings so cur completes ASAP (gates all
                # products)
                nc.scalar.dma_start(
                    out=tM[:, 0:2, 1 : W + 1], in_=cur_blocks[:, 0:2, :]
                )
                nc.sync.dma_start(
                    out=tM[:, 2:NBLK, 1 : W + 1], in_=cur_blocks[:, 2:NBLK, :]
                )

                # coarse is only needed by the center tap in the dy=0 group
                # (processed last) — defer its load past the dy=+1 planes
                tC = coa_pool.tile([128, NBLK, W], fp32, tag="coa")

                aff_flat = aff_d[b].flatten_outer_dims()  # [9*512, 512]

                acc = acc_pool.tile([128, NBLK, W], fp32, tag="acc")
                out_blocks = out_d[b, 0].rearrange("(t p) c -> p t c", p=128)
                psum_tiles = [
                    psum_pool.tile([128, W], fp32, tag="psum", name=f"ps{b}_{t}")
                    for t in range(NBLK)
                ]

                def _evac_store(t, out_ring):
                    nc.scalar.copy(out=acc[:, t, :], in_=psum_tiles[t])
                    out_ring.dma_start(out=out_blocks[:, t, :], in_=acc[:, t, :])

                def _load_group(g, add_eng, ring0, ring1, mul0_eng=None):
                    """Load dy-group g (3 planes, rows shifted -dy) as two
                    half DMAs on two rings, multiply each plane against the
                    shifted cur (or coarse for the center tap), and
                    tree-sum. The final add downcasts to bf16 for the PE.
                    Returns V_g (bf16)."""
                    dy = g - 1
                    start = 1536 * g - dy
                    ag = aff_pool.tile([128, 12, W], fp32, tag="aff")
                    grp = aff_flat[start : start + 1536, :].rearrange(
                        "(t p) c -> p t c", p=128
                    )
                    ring0.dma_start(out=ag[:, 0:6, :], in_=grp[:, 0:6, :])
                    ring1.dma_start(out=ag[:, 6:12, :], in_=grp[:, 6:12, :])
                    Pg = []
                    for j in range(3):
                        k = 3 * g + j
                        dx = j - 1
                        src = tC[:] if k == 4 else tM[:, :, 1 + dx : 1 + dx + W]
                        pk = prod_pool.tile([128, NBLK, W], fp32, tag="prod")
                        meng = mul0_eng if (j == 0 and mul0_eng) else nc.vector
                        meng.tensor_mul(
                            out=pk[:], in0=ag[:, 4 * j : 4 * j + 4, :], in1=src
                        )
                        Pg.append(pk)
                        if j == 1:
                            add_eng.tensor_add(out=Pg[0][:], in0=Pg[0][:], in1=Pg[1][:])
                    V = vsum_pool.tile([128, NBLK, W], bf16, tag="vsum")
                    nc.vector.tensor_add(out=V[:], in0=Pg[0][:], in1=Pg[2][:])
                    return V

                # Groups are processed dy=+1, dy=-1, dy=0: each group's
                # shift-matmuls fire as soon as its sum exists, so by the
                # time the last group (dy=0, plain identity matmuls) lands,
                # the PE queue is nearly drained and the tail is short.
                # psum[t] accumulation order: Su(start), [Eu], Sd, [Ed],
                # I(stop).
                Vp1 = _load_group(2, nc.gpsimd, nc.sync, nc.scalar, mul0_eng=nc.gpsimd)
                nc.scalar.dma_start(
                    out=tC[:], in_=coa_d[b, 0].rearrange("(t p) c -> p t c", p=128)
                )
                if not smats_loaded:
                    nc.scalar.dma_start(out=tS[:], in_=smat_d[:])
                    smats_loaded = True
                for t in range(NBLK):
                    nc.tensor.matmul(
                        psum_tiles[t], SM_SU, Vp1[:, t, :], start=True, stop=False
                    )
                    if t < NBLK - 1:
                        nc.tensor.matmul(
                            psum_tiles[t], SM_EU, Vp1[:, t + 1, :],
                            start=False, stop=False,
                        )

                Vm1 = _load_group(0, nc.gpsimd, nc.scalar, nc.sync, mul0_eng=nc.gpsimd)
                for t in range(NBLK):
                    nc.tensor.matmul(
                        psum_tiles[t], SM_SD, Vm1[:, t, :], start=False, stop=False
                    )
                    if t > 0:
                        nc.tensor.matmul(
                            psum_tiles[t], SM_ED, Vm1[:, t - 1, :],
                            start=False, stop=False,
                        )

                if not last_img:
                    # --- dy = 0 group, whole-plane path ---
                    V0 = _load_group(1, nc.gpsimd, nc.sync, nc.scalar)
                    for t in range(NBLK):
                        nc.tensor.matmul(
                            psum_tiles[t], SM_I, V0[:, t, :], start=False, stop=True
                        )
                        _evac_store(t, nc.scalar)
                else:
                    # --- dy = 0 group for the last image: block-halves.
                    # Half 0 (blocks 0-1) loads via ACT while half 1
                    # (blocks 2-3) loads via SP concurrently; psum[0]/[1]
                    # complete as soon as half 0's sum exists, so their
                    # evacuation and stores overlap half 1's compute. The
                    # final serial chain is half-sized.
                    for h in range(2):
                        ring = nc.scalar if h == 0 else nc.sync
                        Ph = []
                        for j in range(3):
                            k = 3 + j
                            dx = j - 1
                            ak = afft_pool.tile([128, 2, W], fp32, tag="afft")
                            start = 512 * k + 256 * h
                            ring.dma_start(
                                out=ak[:],
                                in_=aff_flat[start : start + 256, :].rearrange(
                                    "(t p) c -> p t c", p=128
                                ),
                            )
                            src = (
                                tC[:, 2 * h : 2 * h + 2, :]
                                if k == 4
                                else tM[:, 2 * h : 2 * h + 2, 1 + dx : 1 + dx + W]
                            )
                            pk = prod_pool.tile([128, 2, W], fp32, tag="prod")
                            nc.vector.tensor_mul(out=pk[:], in0=ak[:], in1=src)
                            Ph.append(pk)
                            if j == 1:
                                nc.vector.tensor_add(
                                    out=Ph[0][:], in0=Ph[0][:], in1=Ph[1][:]
                                )
                        Vh = vsum_pool.tile([128, 2, W], bf16, tag="vsum")
                        nc.vector.tensor_add(out=Vh[:], in0=Ph[0][:], in1=Ph[2][:])
                        for th in range(2):
                            t = 2 * h + th
                            nc.tensor.matmul(
                                psum_tiles[t], SM_I, Vh[:, th, :],
                                start=False, stop=True,
                            )
                            _evac_store(t, nc.scalar if th == 0 else nc.sync)

    nc.compile()
    return nc


def _get_program(reps=1):
    global _compiled
    if reps != 1:
        if reps not in _compiled_reps:
            _compiled_reps[reps] = _build_program(reps)
        return _compiled_reps[reps]
    if _compiled is None:
        _compiled = _build_program()
    return _compiled


def _in_maps(affinity, cur_seg, coarse_seg):
    smats = _shift_mats()
    maps = []
    for j in range(N_CORES):
        s = slice(j * B_PER_CORE, (j + 1) * B_PER_CORE)
        maps.append(
            {
                "affinity": np.ascontiguousarray(affinity[s]),
                "cur_seg": np.ascontiguousarray(cur_seg[s]),
                "coarse_seg": np.ascontiguousarray(coarse_seg[s]),
                "smats": smats,
            }
        )
    return maps


def kernel(affinity, cur_seg, coarse_seg, i=None, **_unused):
    from concourse.bass_utils import run_bass_kernel_spmd

    nc = _get_program()

    affinity = np.ascontiguousarray(affinity, dtype=np.float32)
    cur_seg = np.ascontiguousarray(cur_seg, dtype=np.float32)
    coarse_seg = np.ascontiguousarray(coarse_seg, dtype=np.float32)

    res = run_bass_kernel_spmd(
        nc, _in_maps(affinity, cur_seg, coarse_seg), core_ids=list(range(N_CORES))
    )
    out = np.concatenate([r["out"] for r in res.results], axis=0)
    return out


# revision 7
# speedup vs baseline: 59.5079x; 1.9595x over previous
"""CSPN 3x3 propagation step on 8 Trainium2 NeuronCores.

out[b,0,r,c] = sum_k aff[b,k,r,c] * patch_k(cur)[r,c], with the center tap
(k=4) taken from coarse_seg instead of cur_seg. Zero padding at image edges.

Sharding: pure data parallel over batch (16 images -> 2 per core), one SPMD
Bass program run on all 8 cores with per-core input slices.

Per-core algorithm (per 512x512 image, packed as [128 partitions, 4 row
blocks, 512 cols]):
  - The tap row-shift (dy) is folded into the affinity DMA: plane k is
    loaded with a source row offset of -dy_k (A'_k[s] = aff_k[s-dy]).
    Within a dy-group (3 planes, same dy) the shifted windows are
    contiguous in HBM, so each group loads as ONE 3 MB dma_start
    ([128, 12, 512]; plane j of the group is the [:, 4j:4j+4, :] slice).
    The overhanging first/last source row of the shifted window lands in
    an adjacent affinity plane (never out of bounds) and its product is
    provably never consumed.
  - The tap col-shift (dx) is a free-dim offset into a column-padded cur
    tile.
  - VectorEngine/GpSimd compute the 9 elementwise products P_k = A'_k *
    cur_x with bf16 outputs (the multiply reads fp32 either way, but
    bf16 products let every subsequent add run in the DVE's 2x packed
    mode) and per-dy-group sums V_g in bf16 (rel err ~2e-3, far inside
    the 2e-2 gate; the shift weights are exact in bf16 and PSUM
    accumulates fp32).
  - TensorEngine realigns the dy groups with bf16 shift-matrix matmuls
    (weights exact 0/1) accumulating in fp32 PSUM, including the
    cross-block boundary rows. bf16 operands keep the PE on the fast
    path (fp32 matmul streams at a fraction of bf16 rate).
  - ScalarEngine evacuates PSUM -> SBUF; DMA stores the result.
"""

import sys

import numpy as np

if "/opt/trn_rl_repo" not in sys.path:
    sys.path.insert(0, "/opt/trn_rl_repo")

B_PER_CORE = 2
N_CORES = 8
H = 512
W = 512
NBLK = H // 128
WPAD = W + 2  # zero column on each side

_compiled = None
_compiled_reps = {}


def _shift_mats():
    """[128, 5, 128] bf16: j=0 I, 1 Sd (k=m-1), 2 Su (k=m+1), 3 Ed, 4 Eu."""
    m = np.zeros((128, 5, 128), dtype=np.float32)
    for i in range(128):
        m[i, 0, i] = 1.0  # identity
    for i in range(127):
        m[i, 1, i + 1] = 1.0  # Sd: out[m] = in[m-1]
        m[i + 1, 2, i] = 1.0  # Su: out[m] = in[m+1]
    m[127, 3, 0] = 1.0  # Ed: out[0] = in[127]   (prev block)
    m[0, 4, 127] = 1.0  # Eu: out[127] = in[0]   (next block)
    from concourse import mybir

    return m.astype(mybir.dt.np(mybir.dt.bfloat16))


def _build_program(reps=1):
    """reps>1 unrolls the whole per-core computation `reps` times inside one
    NEFF — used only to measure kernel time through the dispatch noise."""
    import concourse.bacc as bacc
    import concourse.mybir as mybir
    import concourse.tile as tile

    fp32 = mybir.dt.float32
    bf16 = mybir.dt.bfloat16

    nc = bacc.Bacc(
        "TRN2",
        target_bir_lowering=False,
        debug=False,
        enable_asserts=False,
        num_devices=N_CORES,
    )

    aff_d = nc.dram_tensor(
        "affinity", [B_PER_CORE, 9, H, W], fp32, kind="ExternalInput"
    ).ap()
    cur_d = nc.dram_tensor(
        "cur_seg", [B_PER_CORE, 1, H, W], fp32, kind="ExternalInput"
    ).ap()
    coa_d = nc.dram_tensor(
        "coarse_seg", [B_PER_CORE, 1, H, W], fp32, kind="ExternalInput"
    ).ap()
    smat_d = nc.dram_tensor("smats", [128, 5, 128], bf16, kind="ExternalInput").ap()
    out_d = nc.dram_tensor(
        "out", [B_PER_CORE, 1, H, W], fp32, kind="ExternalOutput"
    ).ap()

    with tile.TileContext(nc) as tc:
        with (
            tc.tile_pool(name="smat", bufs=1) as smat_pool,
            tc.tile_pool(name="aff", bufs=2) as aff_pool,
            tc.tile_pool(name="afft", bufs=7) as afft_pool,
            tc.tile_pool(name="prod", bufs=5) as prod_pool,
            tc.tile_pool(name="vsum", bufs=6) as vsum_pool,
            tc.tile_pool(name="cur", bufs=2) as cur_pool,
            tc.tile_pool(name="coa", bufs=2) as coa_pool,
            tc.tile_pool(name="acc", bufs=2) as acc_pool,
            tc.tile_pool(name="psum", bufs=8, space="PSUM") as psum_pool,
        ):
            tS = smat_pool.tile([128, 5, 128], bf16)
            SM_I, SM_SD, SM_SU, SM_ED, SM_EU = (tS[:, j, :] for j in range(5))
            smats_loaded = False

            for b in [bb for _ in range(reps) for bb in range(B_PER_CORE)]:
                last_img = b == B_PER_CORE - 1
                # --- cur tile [128, 4, 514], data in cols 1..512 ---
                # cur/coarse ride the ACT HWDGE ring; affinity groups are
                # split across both rings so the streams overlap.
                tM = cur_pool.tile([128, NBLK, WPAD], fp32, tag="cur")
                nc.vector.memset(tM[:, :, 0:1], 0.0)
                nc.vector.memset(tM[:, :, WPAD - 1 : WPAD], 0.0)
                cur_blocks = cur_d[b, 0].rearrange("(t p) c -> p t c", p=128)
                # split across both rings so cur completes ASAP (gates all
                # products)
                nc.scalar.dma_start(
                    out=tM[:, 0:2, 1 : W + 1], in_=cur_blocks[:, 0:2, :]
                )
                nc.sync.dma_start(
                    out=tM[:, 2:NBLK, 1 : W + 1], in_=cur_blocks[:, 2:NBLK, :]
                )

                # coarse is only needed by the center tap in the dy=0 group
                # (processed last) — defer its load past the dy=+1 planes
                tC = coa_pool.tile([128, NBLK, W], fp32, tag="coa")

                aff_flat = aff_d[b].flatten_outer_dims()  # [9*512, 512]

                acc = acc_pool.tile([128, NBLK, W], fp32, tag="acc")
                out_blocks = out_d[b, 0].rearrange("(t p) c -> p t c", p=128)
                psum_tiles = [
                    psum_pool.tile([128, W], fp32, tag="psum", name=f"ps{b}_{t}")
                    for t in range(NBLK)
                ]

                def _evac_store(t, out_ring):
                    nc.scalar.copy(out=acc[:, t, :], in_=psum_tiles[t])
                    out_ring.dma_start(out=out_blocks[:, t, :], in_=acc[:, t, :])

                def _load_group(g, add_eng, ring0, ring1, mul0_eng=None):
                    """Load dy-group g (3 planes, rows shifted -dy) as two
                    half DMAs on two rings, multiply each plane against the
                    shifted cur (or coarse for the center tap), and
                    tree-sum. The final add downcasts to bf16 for the PE.
                    Returns V_g (bf16)."""
                    dy = g - 1
                    start = 1536 * g - dy
                    ag = aff_pool.tile([128, 12, W], fp32, tag="aff")
                    grp = aff_flat[start : start + 1536, :].rearrange(
                        "(t p) c -> p t c", p=128
                    )
                    ring0.dma_start(out=ag[:, 0:6, :], in_=grp[:, 0:6, :])
                    ring1.dma_start(out=ag[:, 6:12, :], in_=grp[:, 6:12, :])
                    Pg = []
                    for j in range(3):
                        k = 3 * g + j
                        dx = j - 1
                        src = tC[:] if k == 4 else tM[:, :, 1 + dx : 1 + dx + W]
                        pk = prod_pool.tile([128, NBLK, W], bf16, tag="prod")
                        meng = mul0_eng if (j == 0 and mul0_eng) else nc.vector
                        meng.tensor_mul(
                            out=pk[:], in0=ag[:, 4 * j : 4 * j + 4, :], in1=src
                        )
                        Pg.append(pk)
                        if j == 1:
                            add_eng.tensor_add(out=Pg[0][:], in0=Pg[0][:], in1=Pg[1][:])
                    V = vsum_pool.tile([128, NBLK, W], bf16, tag="vsum")
                    nc.vector.tensor_add(out=V[:], in0=Pg[0][:], in1=Pg[2][:])
                    return V

                # Groups are processed dy=+1, dy=-1, dy=0: each group's
                # shift-matmuls fire as soon as its sum exists, so by the
                # time the last group (dy=0, plain identity matmuls) lands,
                # the PE queue is nearly drained and the tail is short.
                # psum[t] accumulation order: Su(start), [Eu], Sd, [Ed],
                # I(stop).
                Vp1 = _load_group(2, nc.gpsimd, nc.sync, nc.scalar, mul0_eng=nc.gpsimd)
                nc.scalar.dma_start(
                    out=tC[:], in_=coa_d[b, 0].rearrange("(t p) c -> p t c", p=128)
                )
                if not smats_loaded:
                    nc.scalar.dma_start(out=tS[:], in_=smat_d[:])
                    smats_loaded = True
                for t in range(NBLK):
                    nc.tensor.matmul(
                        psum_tiles[t], SM_SU, Vp1[:, t, :], start=True, stop=False
                    )
                    if t < NBLK - 1:
                        nc.tensor.matmul(
                            psum_tiles[t], SM_EU, Vp1[:, t + 1, :],
                            start=False, stop=False,
                        )

                Vm1 = _load_group(0, nc.gpsimd, nc.scalar, nc.sync, mul0_eng=nc.gpsimd)
                for t in range(NBLK):
                    nc.tensor.matmul(
                        psum_tiles[t], SM_SD, Vm1[:, t, :], start=False, stop=False
                    )
                    if t > 0:
                        nc.tensor.matmul(
                            psum_tiles[t], SM_ED, Vm1[:, t - 1, :],
                            start=False, stop=False,
                        )

                if not last_img:
                    # --- dy = 0 group, whole-plane path ---
                    V0 = _load_group(1, nc.gpsimd, nc.sync, nc.scalar)
                    for t in range(NBLK):
                        nc.tensor.matmul(
                            psum_tiles[t], SM_I, V0[:, t, :], start=False, stop=True
                        )
                        _evac_store(t, nc.sync)
                else:
                    # --- dy = 0 group for the last image: block-halves.
                    # Half 0 (blocks 0-1) loads via ACT while half 1
                    # (blocks 2-3) loads via SP concurrently; psum[0]/[1]
                    # complete as soon as half 0's sum exists, so their
                    # evacuation and stores overlap half 1's compute. The
                    # final serial chain is half-sized.
                    for h in range(2):
                        ring = nc.scalar if h == 0 else nc.sync
                        Ph = []
                        for j in range(3):
                            k = 3 + j
                            dx = j - 1
                            ak = afft_pool.tile([128, 2, W], fp32, tag="afft")
                            start = 512 * k + 256 * h
                            ring.dma_start(
                                out=ak[:],
                                in_=aff_flat[start : start + 256, :].rearrange(
                                    "(t p) c -> p t c", p=128
                                ),
                            )
                            src = (
                                tC[:, 2 * h : 2 * h + 2, :]
                                if k == 4
                                else tM[:, 2 * h : 2 * h + 2, 1 + dx : 1 + dx + W]
                            )
                            pk = prod_pool.tile([128, 2, W], bf16, tag="prod")
                            nc.vector.tensor_mul(out=pk[:], in0=ak[:], in1=src)
                            Ph.append(pk)
                            if j == 1:
                                nc.vector.tensor_add(
                                    out=Ph[0][:], in0=Ph[0][:], in1=Ph[1][:]
                                )
                        Vh = vsum_pool.tile([128, 2, W], bf16, tag="vsum")
                        nc.vector.tensor_add(out=Vh[:], in0=Ph[0][:], in1=Ph[2][:])
                        for th in range(2):
                            t = 2 * h + th
                            nc.tensor.matmul(
                                psum_tiles[t], SM_I, Vh[:, th, :],
                                start=False, stop=True,
                            )
                            _evac_store(t, nc.scalar if th == 0 else nc.sync)

    nc.compile()
    return nc


def _get_program(reps=1):
    global _compiled
    if reps != 1:
        if reps not in _compiled_reps:
            _compiled_reps[reps] = _build_program(reps)
        return _compiled_reps[reps]
    if _compiled is None:
        _compiled = _build_program()
    return _compiled


def _in_maps(affinity, cur_seg, coarse_seg):
    smats = _shift_mats()
    maps = []
    for j in range(N_CORES):
        s = slice(j * B_PER_CORE, (j + 1) * B_PER_CORE)
        maps.append(
            {
                "affinity": np.ascontiguousarray(affinity[s]),
                "cur_seg": np.ascontiguousarray(cur_seg[s]),
                "coarse_seg": np.ascontiguousarray(coarse_seg[s]),
                "smats": smats,
            }
        )
    return maps


def kernel(affinity, cur_seg, coarse_seg, i=None, **_unused):
    from concourse.bass_utils import run_bass_kernel_spmd

    nc = _get_program()

    affinity = np.ascontiguousarray(affinity, dtype=np.float32)
    cur_seg = np.ascontiguousarray(cur_seg, dtype=np.float32)
    coarse_seg = np.ascontiguousarray(coarse_seg, dtype=np.float32)

    res = run_bass_kernel_spmd(
        nc, _in_maps(affinity, cur_seg, coarse_seg), core_ids=list(range(N_CORES))
    )
    out = np.concatenate([r["out"] for r in res.results], axis=0)
    return out


# revision 9
# speedup vs baseline: 63.4593x; 1.0664x over previous
"""CSPN 3x3 propagation step on 8 Trainium2 NeuronCores.

out[b,0,r,c] = sum_k aff[b,k,r,c] * patch_k(cur)[r,c], with the center tap
(k=4) taken from coarse_seg instead of cur_seg. Zero padding at image edges.

Sharding: pure data parallel over batch (16 images -> 2 per core), one SPMD
Bass program run on all 8 cores with per-core input slices.

Per-core algorithm (per 512x512 image, packed as [128 partitions, 4 row
blocks, 512 cols]):
  - The tap row-shift (dy) is folded into the affinity DMA: plane k is
    loaded with a source row offset of -dy_k (A'_k[s] = aff_k[s-dy]).
    Within a dy-group (3 planes, same dy) the shifted windows are
    contiguous in HBM, so each group loads as ONE 3 MB dma_start
    ([128, 12, 512]; plane j of the group is the [:, 4j:4j+4, :] slice).
    The overhanging first/last source row of the shifted window lands in
    an adjacent affinity plane (never out of bounds) and its product is
    provably never consumed.
  - The tap col-shift (dx) is a free-dim offset into a column-padded cur
    tile.
  - VectorEngine/GpSimd compute the 9 elementwise products P_k = A'_k *
    cur_x with bf16 outputs (the multiply reads fp32 either way, but
    bf16 products let every subsequent add run in the DVE's 2x packed
    mode) and per-dy-group sums V_g in bf16 (rel err ~2e-3, far inside
    the 2e-2 gate; the shift weights are exact in bf16 and PSUM
    accumulates fp32).
  - TensorEngine realigns the dy groups with bf16 shift-matrix matmuls
    (weights exact 0/1) accumulating in fp32 PSUM, including the
    cross-block boundary rows. bf16 operands keep the PE on the fast
    path (fp32 matmul streams at a fraction of bf16 rate).
  - ScalarEngine evacuates PSUM -> SBUF; DMA stores the result.
"""

import sys

import numpy as np

if "/opt/trn_rl_repo" not in sys.path:
    sys.path.insert(0, "/opt/trn_rl_repo")

B_PER_CORE = 2
N_CORES = 8
H = 512
W = 512
NBLK = H // 128
WPAD = W + 2  # zero column on each side

_compiled = None
_compiled_reps = {}


def _shift_mats():
    """[128, 5, 128] bf16: j=0 I, 1 Sd (k=m-1), 2 Su (k=m+1), 3 Ed, 4 Eu."""
    m = np.zeros((128, 5, 128), dtype=np.float32)
    for i in range(128):
        m[i, 0, i] = 1.0  # identity
    for i in range(127):
        m[i, 1, i + 1] = 1.0  # Sd: out[m] = in[m-1]
        m[i + 1, 2, i] = 1.0  # Su: out[m] = in[m+1]
    m[127, 3, 0] = 1.0  # Ed: out[0] = in[127]   (prev block)
    m[0, 4, 127] = 1.0  # Eu: out[127] = in[0]   (next block)
    from concourse import mybir

    return m.astype(mybir.dt.np(mybir.dt.bfloat16))


def _build_program(reps=1):
    """reps>1 unrolls the whole per-core computation `reps` times inside one
    NEFF — used only to measure kernel time through the dispatch noise."""
    import concourse.bacc as bacc
    import concourse.mybir as mybir
    import concourse.tile as tile

    fp32 = mybir.dt.float32
    bf16 = mybir.dt.bfloat16

    nc = bacc.Bacc(
        "TRN2",
        target_bir_lowering=False,
        debug=False,
        enable_asserts=False,
        num_devices=N_CORES,
    )

    aff_d = nc.dram_tensor(
        "affinity", [B_PER_CORE, 9, H, W], fp32, kind="ExternalInput"
    ).ap()
    cur_d = nc.dram_tensor(
        "cur_seg", [B_PER_CORE, 1, H, W], fp32, kind="ExternalInput"
    ).ap()
    coa_d = nc.dram_tensor(
        "coarse_seg", [B_PER_CORE, 1, H, W], fp32, kind="ExternalInput"
    ).ap()
    smat_d = nc.dram_tensor("smats", [128, 5, 128], bf16, kind="ExternalInput").ap()
    out_d = nc.dram_tensor(
        "out", [B_PER_CORE, 1, H, W], fp32, kind="ExternalOutput"
    ).ap()

    with tile.TileContext(nc) as tc:
        with (
            tc.tile_pool(name="smat", bufs=1) as smat_pool,
            tc.tile_pool(name="aff", bufs=2) as aff_pool,
            tc.tile_pool(name="afft", bufs=7) as afft_pool,
            tc.tile_pool(name="prod", bufs=5) as prod_pool,
            tc.tile_pool(name="vsum", bufs=6) as vsum_pool,
            tc.tile_pool(name="cur", bufs=2) as cur_pool,
            tc.tile_pool(name="coa", bufs=2) as coa_pool,
            tc.tile_pool(name="acc", bufs=2) as acc_pool,
            tc.tile_pool(name="psum", bufs=8, space="PSUM") as psum_pool,
        ):
            tS = smat_pool.tile([128, 5, 128], bf16)
            SM_I, SM_SD, SM_SU, SM_ED, SM_EU = (tS[:, j, :] for j in range(5))
            smats_loaded = False

            for b in [bb for _ in range(reps) for bb in range(B_PER_CORE)]:
                last_img = b == B_PER_CORE - 1
                # --- cur tile [128, 4, 514], data in cols 1..512 ---
                # cur/coarse ride the ACT HWDGE ring; affinity groups are
                # split across both rings so the streams overlap.
                tM = cur_pool.tile([128, NBLK, WPAD], fp32, tag="cur")
                nc.vector.memset(tM[:, :, 0:1], 0.0)
                nc.vector.memset(tM[:, :, WPAD - 1 : WPAD], 0.0)
                cur_blocks = cur_d[b, 0].rearrange("(t p) c -> p t c", p=128)
                # split across both rings so cur completes ASAP (gates all
                # products)
                nc.scalar.dma_start(
                    out=tM[:, 0:2, 1 : W + 1], in_=cur_blocks[:, 0:2, :]
                )
                nc.sync.dma_start(
                    out=tM[:, 2:NBLK, 1 : W + 1], in_=cur_blocks[:, 2:NBLK, :]
                )

                # coarse is only needed by the center tap in the dy=0 group
                # (processed last) — defer its load past the dy=+1 planes
                tC = coa_pool.tile([128, NBLK, W], fp32, tag="coa")

                aff_flat = aff_d[b].flatten_outer_dims()  # [9*512, 512]

                acc = acc_pool.tile([128, NBLK, W], fp32, tag="acc")
                out_blocks = out_d[b, 0].rearrange("(t p) c -> p t c", p=128)
                psum_tiles = [
                    psum_pool.tile([128, W], fp32, tag="psum", name=f"ps{b}_{t}")
                    for t in range(NBLK)
                ]

                def _evac_store(t, out_ring):
                    nc.scalar.copy(out=acc[:, t, :], in_=psum_tiles[t])
                    out_ring.dma_start(out=out_blocks[:, t, :], in_=acc[:, t, :])

                def _load_group(g, add_eng, ring0, ring1, mul0_eng=None):
                    """Load dy-group g (3 planes, rows shifted -dy) as two
                    half DMAs on two rings, multiply each plane against the
                    shifted cur (or coarse for the center tap), and
                    tree-sum. The final add downcasts to bf16 for the PE.
                    Returns V_g (bf16)."""
                    dy = g - 1
                    start = 1536 * g - dy
                    ag = aff_pool.tile([128, 12, W], fp32, tag="aff")
                    grp = aff_flat[start : start + 1536, :].rearrange(
                        "(t p) c -> p t c", p=128
                    )
                    ring0.dma_start(out=ag[:, 0:6, :], in_=grp[:, 0:6, :])
                    ring1.dma_start(out=ag[:, 6:12, :], in_=grp[:, 6:12, :])
                    Pg = []
                    for j in range(3):
                        k = 3 * g + j
                        dx = j - 1
                        src = tC[:] if k == 4 else tM[:, :, 1 + dx : 1 + dx + W]
                        pk = prod_pool.tile([128, NBLK, W], bf16, tag="prod")
                        meng = mul0_eng if (j == 0 and mul0_eng) else nc.vector
                        meng.tensor_mul(
                            out=pk[:], in0=ag[:, 4 * j : 4 * j + 4, :], in1=src
                        )
                        Pg.append(pk)
                        if j == 1:
                            add_eng.tensor_add(out=Pg[0][:], in0=Pg[0][:], in1=Pg[1][:])
                    V = vsum_pool.tile([128, NBLK, W], bf16, tag="vsum")
                    nc.vector.tensor_add(out=V[:], in0=Pg[0][:], in1=Pg[2][:])
                    return V

                # Groups are processed dy=+1, dy=-1, dy=0: each group's
                # shift-matmuls fire as soon as its sum exists, so by the
                # time the last group (dy=0, plain identity matmuls) lands,
                # the PE queue is nearly drained and the tail is short.
                # psum[t] accumulation order: Su(start), [Eu], Sd, [Ed],
                # I(stop).
                Vp1 = _load_group(2, nc.gpsimd, nc.sync, nc.scalar, mul0_eng=nc.gpsimd)
                # coarse rides SP: ACT already carries the evacuations, so
                # the ring byte-split is ACT-light to equalize engine busy
                nc.sync.dma_start(
                    out=tC[:], in_=coa_d[b, 0].rearrange("(t p) c -> p t c", p=128)
                )
                if not smats_loaded:
                    nc.scalar.dma_start(out=tS[:], in_=smat_d[:])
                    smats_loaded = True
                for t in range(NBLK):
                    nc.tensor.matmul(
                        psum_tiles[t], SM_SU, Vp1[:, t, :], start=True, stop=False
                    )
                    if t < NBLK - 1:
                        nc.tensor.matmul(
                            psum_tiles[t], SM_EU, Vp1[:, t + 1, :],
                            start=False, stop=False,
                        )

                Vm1 = _load_group(0, nc.gpsimd, nc.scalar, nc.sync, mul0_eng=nc.gpsimd)
                for t in range(NBLK):
                    nc.tensor.matmul(
                        psum_tiles[t], SM_SD, Vm1[:, t, :], start=False, stop=False
                    )
                    if t > 0:
                        nc.tensor.matmul(
                            psum_tiles[t], SM_ED, Vm1[:, t - 1, :],
                            start=False, stop=False,
                        )

                if not last_img:
                    # --- dy = 0 group, whole-plane path ---
                    V0 = _load_group(1, nc.gpsimd, nc.sync, nc.scalar)
                    for t in range(NBLK):
                        nc.tensor.matmul(
                            psum_tiles[t], SM_I, V0[:, t, :], start=False, stop=True
                        )
                        _evac_store(t, nc.scalar if t % 2 == 0 else nc.sync)
                else:
                    # --- dy = 0 group for the last image: block-halves.
                    # Half 0 (blocks 0-1) loads via ACT while half 1
                    # (blocks 2-3) loads via SP concurrently; psum[0]/[1]
                    # complete as soon as half 0's sum exists, so their
                    # evacuation and stores overlap half 1's compute. The
                    # final serial chain is half-sized.
                    for h in range(2):
                        ring = nc.scalar if h == 0 else nc.sync
                        Ph = []
                        for j in range(3):
                            k = 3 + j
                            dx = j - 1
                            ak = afft_pool.tile([128, 2, W], fp32, tag="afft")
                            start = 512 * k + 256 * h
                            ring.dma_start(
                                out=ak[:],
                                in_=aff_flat[start : start + 256, :].rearrange(
                                    "(t p) c -> p t c", p=128
                                ),
                            )
                            src = (
                                tC[:, 2 * h : 2 * h + 2, :]
                                if k == 4
                                else tM[:, 2 * h : 2 * h + 2, 1 + dx : 1 + dx + W]
                            )
                            pk = prod_pool.tile([128, 2, W], bf16, tag="prod")
                            nc.vector.tensor_mul(out=pk[:], in0=ak[:], in1=src)
                            Ph.append(pk)
                            if j == 1:
                                nc.vector.tensor_add(
                                    out=Ph[0][:], in0=Ph[0][:], in1=Ph[1][:]
                                )
                        Vh = vsum_pool.tile([128, 2, W], bf16, tag="vsum")
                        nc.vector.tensor_add(out=Vh[:], in0=Ph[0][:], in1=Ph[2][:])
                        for th in range(2):
                            t = 2 * h + th
                            nc.tensor.matmul(
                                psum_tiles[t], SM_I, Vh[:, th, :],
                                start=False, stop=True,
                            )
                            _evac_store(t, nc.scalar if th == 0 else nc.sync)

    nc.compile()
    return nc


def _get_program(reps=1):
    global _compiled
    if reps != 1:
        if reps not in _compiled_reps:
            _compiled_reps[reps] = _build_program(reps)
        return _compiled_reps[reps]
    if _compiled is None:
        _compiled = _build_program()
    return _compiled


def _in_maps(affinity, cur_seg, coarse_seg):
    smats = _shift_mats()
    maps = []
    for j in range(N_CORES):
        s = slice(j * B_PER_CORE, (j + 1) * B_PER_CORE)
        maps.append(
            {
                "affinity": np.ascontiguousarray(affinity[s]),
                "cur_seg": np.ascontiguousarray(cur_seg[s]),
                "coarse_seg": np.ascontiguousarray(coarse_seg[s]),
                "smats": smats,
            }
        )
    return maps


def kernel(affinity, cur_seg, coarse_seg, i=None, **_unused):
    from concourse.bass_utils import run_bass_kernel_spmd

    nc = _get_program()

    affinity = np.ascontiguousarray(affinity, dtype=np.float32)
    cur_seg = np.ascontiguousarray(cur_seg, dtype=np.float32)
    coarse_seg = np.ascontiguousarray(coarse_seg, dtype=np.float32)

    res = run_bass_kernel_spmd(
        nc, _in_maps(affinity, cur_seg, coarse_seg), core_ids=list(range(N_CORES))
    )
    out = np.concatenate([r["out"] for r in res.results], axis=0)
    return out
